# revision 14
# baseline (speedup 1.0000x reference)
"""CurveEval (NURBS curve evaluation) Trainium2 kernel.

Math: out[b, s, :] = (sum_j basis[s,j] * cp[b, span[s]-3+j, 0:3])
                   / (sum_j basis[s,j] * cp[b, span[s]-3+j, 3])

Strategy:
  - Host: fold (span, basis) into a dense weight matrix W[s, n] with 4
    nonzeros per row; the gather+weighted-sum becomes curves = W @ cp[b],
    batched over b.  W^T [64, 2048] is tiny and replicated to all cores.
  - Shard control_points (batch 4096) across 8 cores, 512 batches each.
  - Precision: the PE's fast fp32 path (float32r) is TF32 (10-bit
    mantissa).  Split both operands into tf32 hi+lo on the host (exact
    bit truncation) and stack hi over lo along the contraction dim
    (K=64 -> 128).  Two accumulating fp32r matmuls per tile
    ([Chi;Clo] @ [Whi;Whi] + [Chi;Clo] @ [Wlo;Wlo]) produce all four
    cross terms: ~2^-21 input representation error at full PE rate.
  - Fast path: spans are sorted, so each 512-sample chunk touches a
    <=32-row window of control points.  With hi/lo rows interleaved and
    the window duplicated across the two K-halves (SBUF->SBUF DMA), all
    four tf32 cross products collapse into ONE K=128 matmul per
    (batch-tile, chunk, channel) - half the PE streaming of the generic
    2-pass scheme.  Falls back to the generic kernel when a chunk's span
    range exceeds the window.
  - Device: for each 128-batch tile and 512-sample chunk, x/y/z/w planes
    to PSUM; reciprocal of the w plane via exp(-ln(w)) on the ACT
    engine; 3 tensor_muls on DVE write the interleaved [b, (s,c)] SBUF
    tile which stores to HBM as fully contiguous 24KiB-per-partition
    DMAs.
"""

import numpy as np

BATCH = 4096
NCTRL = 64
ORDER = 3
S = 2048
DIM = 3
CH = DIM + 1
NCORES = 8
BLOCAL = BATCH // NCORES  # 512
BTILE = 128
SCHUNK = 512
N_BTILES = BLOCAL // BTILE  # 4
N_SCHUNKS = S // SCHUNK  # 4
# merged input layout (columns of the single [128, 6144] input tensor)
CP_COLS = CH * BLOCAL  # 2048
W_COLS = S  # 2048
INS_COLS = CP_COLS + 2 * W_COLS  # 6144

_CACHE = {}


def _tf32_trunc(x):
    return (x.view(np.uint32) & np.uint32(0xFFFFE000)).view(np.float32)


def _tf32_split(x):
    """x (fp32) -> (hi, lo) tf32-representable with hi+lo = x to ~2^-21."""
    x = np.ascontiguousarray(x, dtype=np.float32)
    hi = _tf32_trunc(x)
    lo = _tf32_trunc(x - hi)
    return hi, lo


def _build_bass(fast):
    import concourse.bacc as bacc
    import concourse.mybir as mybir
    from concourse.tile import TileContext

    f32 = mybir.dt.float32
    f32r = mybir.dt.float32r
    AF = mybir.ActivationFunctionType

    nc = bacc.Bacc()

    # Make Ln/Exp/Copy resolve to the single combined act-func table set so
    # the ACT engine loads one table once instead of thrashing between the
    # ln-only and exp-only sets (1.28us per reload, 24 reloads = 31us).
    # get_activation_tables is functools.cache'd; in-place mutation keeps
    # dict order (= act_func_set_id) intact.
    import concourse.hw_specs as hw_specs

    tabs = hw_specs.get_activation_tables(nc.m.arch)
    combo = "natural_log_exp_and_others"
    if combo in tabs:
        steal = {AF.Ln, AF.Exp, AF.Copy, AF.Identity} & tabs[combo]
        for name, fset in tabs.items():
            if name != combo:
                fset -= steal
    if fast:
        # cwin[sc, 2k+e, col] = tf32 hi (e=0) / lo (e=1) of control-point row
        # (r0[sc]+k) at column (bt*512 + c*128 + b); wwin rows: [0:64] = Whi
        # window rows duplicated pairwise, [64:128] = Wlo likewise.
        cwin = nc.dram_tensor(
            "cwin", [N_SCHUNKS, 2 * NCTRL, CP_COLS], f32r, kind="ExternalInput"
        )
        wwin = nc.dram_tensor("wwin", [2 * NCTRL, W_COLS], f32r, kind="ExternalInput")
    else:
        # cpS[hi n (64); lo n (64)] x [bt*512 + c*128 + b_local]
        cpS = nc.dram_tensor("cpS", [2 * NCTRL, CP_COLS], f32r, kind="ExternalInput")
        wS1 = nc.dram_tensor("wS1", [2 * NCTRL, W_COLS], f32r, kind="ExternalInput")
        wS2 = nc.dram_tensor("wS2", [2 * NCTRL, W_COLS], f32r, kind="ExternalInput")
    out = nc.dram_tensor("out", [BLOCAL, S, DIM], f32, kind="ExternalOutput")

    with TileContext(nc) as tc:
        with (
            tc.tile_pool(name="const", bufs=1) as constp,
            tc.tile_pool(name="outp", bufs=6) as outp,
            tc.tile_pool(name="rec", bufs=3) as recp,
            tc.tile_pool(name="psum", bufs=2, space="PSUM") as psp,
        ):
            # fine-grained input loads: first-needed tiles land early so
            # the PE starts quickly and HAM warms up sooner
            if fast:
                cwt, wwt = [], []
                # Loads issue from the ACT sequencer (HWDGE ring separate
                # from SP's) in first-needed order; windows arrive
                # host-duplicated so every load is a full-128-partition
                # (full-bandwidth) transfer.  Chunk 0 loads its first
                # bt-block alone so the first matmul starts right after the
                # preamble.
                b0 = CH * BTILE
                cw0 = constp.tile([2 * NCTRL, CP_COLS], f32r, name="cw_0")
                ww0 = constp.tile([2 * NCTRL, SCHUNK], f32r, name="ww_0")
                nc.scalar.dma_start(out=cw0[:, 0:b0], in_=cwin[0][:, 0:b0])
                nc.scalar.dma_start(out=ww0, in_=wwin[:, 0:SCHUNK])
                nc.scalar.dma_start(out=cw0[:, b0:], in_=cwin[0][:, b0:])
                cwt, wwt = [cw0], [ww0]
                for k in range(1, N_SCHUNKS):
                    cw = constp.tile([2 * NCTRL, CP_COLS], f32r, name=f"cw_{k}")
                    nc.scalar.dma_start(out=cw, in_=cwin[k])
                    cwt.append(cw)
                    ww = constp.tile([2 * NCTRL, SCHUNK], f32r, name=f"ww_{k}")
                    nc.scalar.dma_start(
                        out=ww, in_=wwin[:, k * SCHUNK : (k + 1) * SCHUNK]
                    )
                    wwt.append(ww)
            else:
                cpt, w1t, w2t = [], [], []
                for k in range(N_SCHUNKS):
                    cpb = constp.tile(
                        [2 * NCTRL, CH * BTILE], f32r, name=f"cp_{k}"
                    )
                    nc.scalar.dma_start(
                        out=cpb, in_=cpS[:, k * CH * BTILE : (k + 1) * CH * BTILE]
                    )
                    cpt.append(cpb)
                    w1 = constp.tile([2 * NCTRL, SCHUNK], f32r, name=f"w1_{k}")
                    nc.scalar.dma_start(
                        out=w1, in_=wS1[:, k * SCHUNK : (k + 1) * SCHUNK]
                    )
                    w1t.append(w1)
                    w2 = constp.tile([2 * NCTRL, SCHUNK], f32r, name=f"w2_{k}")
                    nc.scalar.dma_start(
                        out=w2, in_=wS2[:, k * SCHUNK : (k + 1) * SCHUNK]
                    )
                    w2t.append(w2)

            for bt in range(N_BTILES):
                for sc in range(N_SCHUNKS):
                    ot = outp.tile(
                        [BTILE, SCHUNK, DIM], f32, tag="ot", name=f"ot_{bt}_{sc}"
                    )
                    ps = [
                        psp.tile(
                            [BTILE, SCHUNK], f32, tag=f"ps{c}", name=f"ps{c}_{bt}_{sc}"
                        )
                        for c in range(CH)
                    ]
                    for c in range(CH):
                        if fast:
                            lhsT = cwt[sc][
                                :,
                                bt * CH * BTILE + c * BTILE : bt * CH * BTILE
                                + (c + 1) * BTILE,
                            ]
                            nc.tensor.matmul(
                                ps[c], lhsT, wwt[sc], start=True, stop=True
                            )
                        else:
                            lhsT = cpt[bt][:, c * BTILE : (c + 1) * BTILE]
                            nc.tensor.matmul(
                                ps[c], lhsT, w1t[sc], start=True, stop=False
                            )
                            nc.tensor.matmul(
                                ps[c], lhsT, w2t[sc], start=False, stop=True
                            )
                    # recip = 1/w via exp(-ln(w)) on the ACT engine (DVE-free)
                    lnw = recp.tile(
                        [BTILE, SCHUNK], f32, tag="lnw", name=f"ln_{bt}_{sc}"
                    )
                    nc.scalar.activation(out=lnw, in_=ps[DIM], func=AF.Ln)
                    rec = recp.tile(
                        [BTILE, SCHUNK], f32, tag="rec", name=f"rc_{bt}_{sc}"
                    )
                    nc.scalar.activation(out=rec, in_=lnw, func=AF.Exp, scale=-1.0)
                    zsb = recp.tile(
                        [BTILE, SCHUNK], f32, tag="zsb", name=f"zs_{bt}_{sc}"
                    )
                    nc.scalar.copy(out=zsb, in_=ps[2])
                    for c in range(2):
                        nc.vector.tensor_mul(ot[:, :, c], ps[c], rec)
                    nc.gpsimd.tensor_mul(ot[:, :, 2], zsb, rec)
                    # store each finished chunk immediately (0.75 MiB) so the
                    # HBM write stream starts early and stays busy
                    nc.sync.dma_start(
                        out=out[
                            bt * BTILE : (bt + 1) * BTILE,
                            sc * SCHUNK : (sc + 1) * SCHUNK,
                            :,
                        ],
                        in_=ot,
                    )
    # bacc legalization: splits multi-sem waits (HW allows 1 per inst),
    # moves matmul waits to ldweights, event-sem conversion, reg alloc.
    nc.compile()
    return nc


def _get_nc(fast):
    key = "nc_fast" if fast else "nc_safe"
    if key not in _CACHE:
        _CACHE[key] = _build_bass(fast)
    return _CACHE[key]


def _prep_inputs(control_points, span, basis):
    cp = np.ascontiguousarray(np.asarray(control_points, dtype=np.float32))
    sp = np.asarray(span, dtype=np.int64).ravel()
    bs = np.asarray(basis, dtype=np.float32)
    assert cp.shape == (BATCH, NCTRL, CH), cp.shape
    assert sp.shape == (S,), sp.shape
    assert bs.shape == (S, ORDER + 1), bs.shape

    wT = np.zeros((NCTRL, S), dtype=np.float32)
    cols = np.arange(S)
    for j in range(ORDER + 1):
        rows = (sp - ORDER + j) % NCTRL  # python-style wrap, matches jnp
        np.add.at(wT, (rows, cols), bs[:, j])
    whi, wlo = _tf32_split(wT)

    # fast path: per chunk, the (sorted) spans touch control rows
    # [min-ORDER, max]; if that window fits in 32 rows everywhere we can
    # use the single-matmul kernel.
    WIN = NCTRL // 2  # 32
    r0s = []
    fast = True
    for sc in range(N_SCHUNKS):
        ss = sp[sc * SCHUNK : (sc + 1) * SCHUNK]
        lo_ = int(ss.min()) - ORDER
        hi_ = int(ss.max())
        if hi_ - lo_ + 1 > WIN or lo_ < 0 or hi_ >= NCTRL:
            fast = False
            break
        r0s.append(max(0, min(lo_, NCTRL - WIN)))

    if fast:
        wwin = np.zeros((2 * NCTRL, S), dtype=np.float32)
        for sc, r0 in enumerate(r0s):
            blk = slice(sc * SCHUNK, (sc + 1) * SCHUNK)
            idx = r0 + np.arange(WIN)
            wwin[0 : 2 * WIN : 2, blk] = whi[idx][:, blk]
            wwin[1 : 2 * WIN : 2, blk] = whi[idx][:, blk]
            wwin[2 * WIN :: 2, blk] = wlo[idx][:, blk]
            wwin[2 * WIN + 1 :: 2, blk] = wlo[idx][:, blk]
        wwin = np.ascontiguousarray(wwin)
    else:
        wS1 = np.ascontiguousarray(np.concatenate([whi, whi], axis=0))
        wS2 = np.ascontiguousarray(np.concatenate([wlo, wlo], axis=0))

    in_maps = []
    for core in range(NCORES):
        shard = cp[core * BLOCAL : (core + 1) * BLOCAL]  # [512, 64, 4]
        # [n, c, b] -> [n, bt, c, b_local] -> [n, bt*512 + c*128 + b_local]
        a = shard.transpose(1, 2, 0).reshape(NCTRL, CH, N_BTILES, BTILE)
        a = np.ascontiguousarray(a.transpose(0, 2, 1, 3)).reshape(NCTRL, CP_COLS)
        chi, clo = _tf32_split(a)
        if fast:
            cwin = np.empty((N_SCHUNKS, 2 * NCTRL, CP_COLS), dtype=np.float32)
            for sc, r0 in enumerate(r0s):
                idx = r0 + np.arange(WIN)
                cwin[sc, 0:NCTRL:2] = chi[idx]
                cwin[sc, 1:NCTRL:2] = clo[idx]
                cwin[sc, NCTRL:] = cwin[sc, :NCTRL]
            in_maps.append({"cwin": np.ascontiguousarray(cwin), "wwin": wwin})
        else:
            cpS = np.ascontiguousarray(np.concatenate([chi, clo], axis=0))
            in_maps.append({"cpS": cpS, "wS1": wS1, "wS2": wS2})
    return in_maps, fast


def _execute(in_maps, fast, **run_kwargs):
    from concourse.bass_utils import run_bass_kernel_spmd

    nc = _get_nc(fast)
    return run_bass_kernel_spmd(
        nc, in_maps, core_ids=list(range(NCORES)), **run_kwargs
    )


def kernel(control_points, span, basis):
    in_maps, fast = _prep_inputs(control_points, span, basis)
    res = _execute(in_maps, fast)
    return np.concatenate([r["out"] for r in res.results], axis=0)


# revision 15
# speedup vs baseline: 1.1780x; 1.1780x over previous
"""CurveEval (NURBS curve evaluation) Trainium2 kernel.

Math: out[b, s, :] = (sum_j basis[s,j] * cp[b, span[s]-3+j, 0:3])
                   / (sum_j basis[s,j] * cp[b, span[s]-3+j, 3])

Strategy:
  - Host: fold (span, basis) into a dense weight matrix W[s, n] with 4
    nonzeros per row; the gather+weighted-sum becomes curves = W @ cp[b],
    batched over b.  W^T [64, 2048] is tiny and replicated to all cores.
  - Shard control_points (batch 4096) across 8 cores, 512 batches each.
  - Precision: the PE's fast fp32 path (float32r) is TF32 (10-bit
    mantissa).  Split both operands into tf32 hi+lo on the host (exact
    bit truncation) and stack hi over lo along the contraction dim
    (K=64 -> 128).  Two accumulating fp32r matmuls per tile
    ([Chi;Clo] @ [Whi;Whi] + [Chi;Clo] @ [Wlo;Wlo]) produce all four
    cross terms: ~2^-21 input representation error at full PE rate.
  - Fast path: spans are sorted, so each 512-sample chunk touches a
    <=32-row window of control points.  With hi/lo rows interleaved and
    the window duplicated across the two K-halves (SBUF->SBUF DMA), all
    four tf32 cross products collapse into ONE K=128 matmul per
    (batch-tile, chunk, channel) - half the PE streaming of the generic
    2-pass scheme.  Falls back to the generic kernel when a chunk's span
    range exceeds the window.
  - Device: for each 128-batch tile and 512-sample chunk, x/y/z/w planes
    to PSUM; reciprocal of the w plane via exp(-ln(w)) on the ACT
    engine; 3 tensor_muls on DVE write the interleaved [b, (s,c)] SBUF
    tile which stores to HBM as fully contiguous 24KiB-per-partition
    DMAs.
"""

import numpy as np

BATCH = 4096
NCTRL = 64
ORDER = 3
S = 2048
DIM = 3
CH = DIM + 1
NCORES = 8
BLOCAL = BATCH // NCORES  # 512
BTILE = 128
SCHUNK = 512
N_BTILES = BLOCAL // BTILE  # 4
N_SCHUNKS = S // SCHUNK  # 4
# merged input layout (columns of the single [128, 6144] input tensor)
CP_COLS = CH * BLOCAL  # 2048
W_COLS = S  # 2048
INS_COLS = CP_COLS + 2 * W_COLS  # 6144

_CACHE = {}


def _tf32_trunc(x):
    return (x.view(np.uint32) & np.uint32(0xFFFFE000)).view(np.float32)


def _tf32_split(x):
    """x (fp32) -> (hi, lo) tf32-representable with hi+lo = x to ~2^-21."""
    x = np.ascontiguousarray(x, dtype=np.float32)
    hi = _tf32_trunc(x)
    lo = _tf32_trunc(x - hi)
    return hi, lo


def _build_bass(fast):
    import concourse.bacc as bacc
    import concourse.mybir as mybir
    from concourse.tile import TileContext

    f32 = mybir.dt.float32
    f32r = mybir.dt.float32r
    AF = mybir.ActivationFunctionType

    nc = bacc.Bacc()

    # Make Ln/Exp/Copy resolve to the single combined act-func table set so
    # the ACT engine loads one table once instead of thrashing between the
    # ln-only and exp-only sets (1.28us per reload, 24 reloads = 31us).
    # get_activation_tables is functools.cache'd; in-place mutation keeps
    # dict order (= act_func_set_id) intact.
    import concourse.hw_specs as hw_specs

    tabs = hw_specs.get_activation_tables(nc.m.arch)
    combo = "natural_log_exp_and_others"
    if combo in tabs:
        steal = {AF.Ln, AF.Exp, AF.Copy, AF.Identity} & tabs[combo]
        for name, fset in tabs.items():
            if name != combo:
                fset -= steal
    if fast:
        # cwin[sc, 2k+e, col] = tf32 hi (e=0) / lo (e=1) of control-point row
        # (r0[sc]+k) at column (bt*512 + c*128 + b); wwin rows: [0:64] = Whi
        # window rows duplicated pairwise, [64:128] = Wlo likewise.
        cwin = nc.dram_tensor(
            "cwin", [N_SCHUNKS, 2 * NCTRL, CP_COLS], f32r, kind="ExternalInput"
        )
        wwin = nc.dram_tensor("wwin", [2 * NCTRL, W_COLS], f32r, kind="ExternalInput")
    else:
        # cpS[hi n (64); lo n (64)] x [bt*512 + c*128 + b_local]
        cpS = nc.dram_tensor("cpS", [2 * NCTRL, CP_COLS], f32r, kind="ExternalInput")
        wS1 = nc.dram_tensor("wS1", [2 * NCTRL, W_COLS], f32r, kind="ExternalInput")
        wS2 = nc.dram_tensor("wS2", [2 * NCTRL, W_COLS], f32r, kind="ExternalInput")
    out = nc.dram_tensor("out", [BLOCAL, S, DIM], f32, kind="ExternalOutput")

    with TileContext(nc) as tc:
        with (
            tc.tile_pool(name="const", bufs=1) as constp,
            tc.tile_pool(name="outp", bufs=6) as outp,
            tc.tile_pool(name="rec", bufs=3) as recp,
            tc.tile_pool(name="psum", bufs=2, space="PSUM") as psp,
        ):
            # fine-grained input loads: first-needed tiles land early so
            # the PE starts quickly and HAM warms up sooner
            if fast:
                cwt, wwt = [], []
                # Loads issue from the ACT sequencer (HWDGE ring separate
                # from SP's) in first-needed order; windows arrive
                # host-duplicated so every load is a full-128-partition
                # (full-bandwidth) transfer.  Chunk 0 loads its first
                # bt-block alone so the first matmul starts right after the
                # preamble.
                b0 = CH * BTILE
                cw0 = constp.tile([2 * NCTRL, CP_COLS], f32r, name="cw_0")
                ww0 = constp.tile([2 * NCTRL, SCHUNK], f32r, name="ww_0")
                nc.scalar.dma_start(out=cw0[:, 0:b0], in_=cwin[0][:, 0:b0])
                nc.scalar.dma_start(out=ww0, in_=wwin[:, 0:SCHUNK])
                nc.scalar.dma_start(out=cw0[:, b0:], in_=cwin[0][:, b0:])
                cwt, wwt = [cw0], [ww0]
                for k in range(1, N_SCHUNKS):
                    cw = constp.tile([2 * NCTRL, CP_COLS], f32r, name=f"cw_{k}")
                    nc.scalar.dma_start(out=cw, in_=cwin[k])
                    cwt.append(cw)
                    ww = constp.tile([2 * NCTRL, SCHUNK], f32r, name=f"ww_{k}")
                    nc.scalar.dma_start(
                        out=ww, in_=wwin[:, k * SCHUNK : (k + 1) * SCHUNK]
                    )
                    wwt.append(ww)
            else:
                cpt, w1t, w2t = [], [], []
                for k in range(N_SCHUNKS):
                    cpb = constp.tile(
                        [2 * NCTRL, CH * BTILE], f32r, name=f"cp_{k}"
                    )
                    nc.scalar.dma_start(
                        out=cpb, in_=cpS[:, k * CH * BTILE : (k + 1) * CH * BTILE]
                    )
                    cpt.append(cpb)
                    w1 = constp.tile([2 * NCTRL, SCHUNK], f32r, name=f"w1_{k}")
                    nc.scalar.dma_start(
                        out=w1, in_=wS1[:, k * SCHUNK : (k + 1) * SCHUNK]
                    )
                    w1t.append(w1)
                    w2 = constp.tile([2 * NCTRL, SCHUNK], f32r, name=f"w2_{k}")
                    nc.scalar.dma_start(
                        out=w2, in_=wS2[:, k * SCHUNK : (k + 1) * SCHUNK]
                    )
                    w2t.append(w2)

            # sc-outer: chunk k's weights are first needed at unit 4k, so
            # the cw_k load (arriving ~3us apart) is always ahead of the PE
            for sc in range(N_SCHUNKS):
                for bt in range(N_BTILES):
                    ot = outp.tile(
                        [BTILE, SCHUNK, DIM], f32, tag="ot", name=f"ot_{bt}_{sc}"
                    )
                    ps = [
                        psp.tile(
                            [BTILE, SCHUNK], f32, tag=f"ps{c}", name=f"ps{c}_{bt}_{sc}"
                        )
                        for c in range(CH)
                    ]
                    for c in range(CH):
                        if fast:
                            lhsT = cwt[sc][
                                :,
                                bt * CH * BTILE + c * BTILE : bt * CH * BTILE
                                + (c + 1) * BTILE,
                            ]
                            nc.tensor.matmul(
                                ps[c], lhsT, wwt[sc], start=True, stop=True
                            )
                        else:
                            lhsT = cpt[bt][:, c * BTILE : (c + 1) * BTILE]
                            nc.tensor.matmul(
                                ps[c], lhsT, w1t[sc], start=True, stop=False
                            )
                            nc.tensor.matmul(
                                ps[c], lhsT, w2t[sc], start=False, stop=True
                            )
                    # recip = 1/w via exp(-ln(w)) on the ACT engine (DVE-free)
                    lnw = recp.tile(
                        [BTILE, SCHUNK], f32, tag="lnw", name=f"ln_{bt}_{sc}"
                    )
                    nc.scalar.activation(out=lnw, in_=ps[DIM], func=AF.Ln)
                    rec = recp.tile(
                        [BTILE, SCHUNK], f32, tag="rec", name=f"rc_{bt}_{sc}"
                    )
                    nc.scalar.activation(out=rec, in_=lnw, func=AF.Exp, scale=-1.0)
                    zsb = recp.tile(
                        [BTILE, SCHUNK], f32, tag="zsb", name=f"zs_{bt}_{sc}"
                    )
                    nc.scalar.copy(out=zsb, in_=ps[2])
                    for c in range(2):
                        nc.vector.tensor_mul(ot[:, :, c], ps[c], rec)
                    nc.gpsimd.tensor_mul(ot[:, :, 2], zsb, rec)
                    # store each finished chunk immediately (0.75 MiB) so the
                    # HBM write stream starts early and stays busy
                    nc.sync.dma_start(
                        out=out[
                            bt * BTILE : (bt + 1) * BTILE,
                            sc * SCHUNK : (sc + 1) * SCHUNK,
                            :,
                        ],
                        in_=ot,
                    )
    # bacc legalization: splits multi-sem waits (HW allows 1 per inst),
    # moves matmul waits to ldweights, event-sem conversion, reg alloc.
    nc.compile()
    return nc


def _get_nc(fast):
    key = "nc_fast" if fast else "nc_safe"
    if key not in _CACHE:
        _CACHE[key] = _build_bass(fast)
    return _CACHE[key]


def _prep_inputs(control_points, span, basis):
    cp = np.ascontiguousarray(np.asarray(control_points, dtype=np.float32))
    sp = np.asarray(span, dtype=np.int64).ravel()
    bs = np.asarray(basis, dtype=np.float32)
    assert cp.shape == (BATCH, NCTRL, CH), cp.shape
    assert sp.shape == (S,), sp.shape
    assert bs.shape == (S, ORDER + 1), bs.shape

    wT = np.zeros((NCTRL, S), dtype=np.float32)
    cols = np.arange(S)
    for j in range(ORDER + 1):
        rows = (sp - ORDER + j) % NCTRL  # python-style wrap, matches jnp
        np.add.at(wT, (rows, cols), bs[:, j])
    whi, wlo = _tf32_split(wT)

    # fast path: per chunk, the (sorted) spans touch control rows
    # [min-ORDER, max]; if that window fits in 32 rows everywhere we can
    # use the single-matmul kernel.
    WIN = NCTRL // 2  # 32
    r0s = []
    fast = True
    for sc in range(N_SCHUNKS):
        ss = sp[sc * SCHUNK : (sc + 1) * SCHUNK]
        lo_ = int(ss.min()) - ORDER
        hi_ = int(ss.max())
        if hi_ - lo_ + 1 > WIN or lo_ < 0 or hi_ >= NCTRL:
            fast = False
            break
        r0s.append(max(0, min(lo_, NCTRL - WIN)))

    if fast:
        wwin = np.zeros((2 * NCTRL, S), dtype=np.float32)
        for sc, r0 in enumerate(r0s):
            blk = slice(sc * SCHUNK, (sc + 1) * SCHUNK)
            idx = r0 + np.arange(WIN)
            wwin[0 : 2 * WIN : 2, blk] = whi[idx][:, blk]
            wwin[1 : 2 * WIN : 2, blk] = whi[idx][:, blk]
            wwin[2 * WIN :: 2, blk] = wlo[idx][:, blk]
            wwin[2 * WIN + 1 :: 2, blk] = wlo[idx][:, blk]
        wwin = np.ascontiguousarray(wwin)
    else:
        wS1 = np.ascontiguousarray(np.concatenate([whi, whi], axis=0))
        wS2 = np.ascontiguousarray(np.concatenate([wlo, wlo], axis=0))

    in_maps = []
    for core in range(NCORES):
        shard = cp[core * BLOCAL : (core + 1) * BLOCAL]  # [512, 64, 4]
        # [n, c, b] -> [n, bt, c, b_local] -> [n, bt*512 + c*128 + b_local]
        a = shard.transpose(1, 2, 0).reshape(NCTRL, CH, N_BTILES, BTILE)
        a = np.ascontiguousarray(a.transpose(0, 2, 1, 3)).reshape(NCTRL, CP_COLS)
        chi, clo = _tf32_split(a)
        if fast:
            cwin = np.empty((N_SCHUNKS, 2 * NCTRL, CP_COLS), dtype=np.float32)
            for sc, r0 in enumerate(r0s):
                idx = r0 + np.arange(WIN)
                cwin[sc, 0:NCTRL:2] = chi[idx]
                cwin[sc, 1:NCTRL:2] = clo[idx]
                cwin[sc, NCTRL:] = cwin[sc, :NCTRL]
            in_maps.append({"cwin": np.ascontiguousarray(cwin), "wwin": wwin})
        else:
            cpS = np.ascontiguousarray(np.concatenate([chi, clo], axis=0))
            in_maps.append({"cpS": cpS, "wS1": wS1, "wS2": wS2})
    return in_maps, fast


def _execute(in_maps, fast, **run_kwargs):
    from concourse.bass_utils import run_bass_kernel_spmd

    nc = _get_nc(fast)
    return run_bass_kernel_spmd(
        nc, in_maps, core_ids=list(range(NCORES)), **run_kwargs
    )


def kernel(control_points, span, basis):
    in_maps, fast = _prep_inputs(control_points, span, basis)
    res = _execute(in_maps, fast)
    return np.concatenate([r["out"] for r in res.results], axis=0)
